# revision 18
# baseline (speedup 1.0000x reference)
"""Multi-head causal attention (RoPE) on 8 TRN2 NeuronCores.

Sharding: tensor-parallel over heads. Each core computes 2 of the 16 heads:
column-parallel q/k/v projections, local attention, then a per-batch-row
AllToAll of the transposed attention outputs and a token-parallel o-proj
(each core produces the full 1024-wide output for 128 tokens per row).

Layout strategy: activations live transposed on-chip ([dim, token]) so every
matmul contracts over the partition axis with no transposes of x. Scores are
computed transposed ([tk, tq]); softmax has no max-subtraction (logits are
O(1) for this input distribution) and its denominator is produced by a
64-wide ones block appended to V in the PV matmul; normalization is a single
tensor-tensor divide per (b, head, tq-half) writing bf16 aoT directly.
RoPE uses the interleaved-pair identity q' = q*C + swap(q)*S', with the pair
swap done by the DVE stream-shuffle.

e-tiles and v-tiles are double-buffered by row parity so next-row filler
writes never WAR-stall against the current row's PV reads.

o-proj is token-stationary: after the per-row AllToAll each core holds all
1024 attention dims for its 128 tokens of that row; the 128-token tile is the
matmul stationary operand and Wo.T streams as the moving operand (N=512).

Schedule: attention(b) is emitted with next-row QKV projection units
INTERLEAVED between its score/PV groups (keeps the in-order PE queue fed
behind exp-gated groups). The aoT->ag_in bounce is split per tq-half and
issued as each half's normalize completes, pulling the collective trigger
earlier. ALL o-proj compute is deferred until after the A2A(3) trigger so
the last collective's straggler wait is filled with oproj(0..2); oproj(3)
starts right after A2A(3) lands via per-ct aof gathers split across two
otherwise-idle DMA rings. dma_start BLOCKS its issuing engine until the
transfer's input semaphore is live, so every aof gather is placed on an
engine/queue position where that block is harmless, and all yo drains run
on the vector engine (scalar sits blocked behind aof gathers at the tail).
x is host-packed dense and th-major on SBUF so every x DMA is a fully
linear 8-16KB-per-partition transfer.
"""

import sys

for _p in ("/opt/trn_rl_repo",):
    if _p not in sys.path:
        sys.path.insert(0, _p)

import contextlib

import numpy as np
import ml_dtypes

import concourse.bass as bass
import concourse.mybir as mybir
import concourse.tile as tile
from concourse import bacc
from concourse.bass_utils import run_bass_kernel_spmd
from concourse.masks import make_identity

# Problem constants (nn_MultiHeadAttention: x [4,1024,1024], 16 heads)
B, T, D = 4, 1024, 1024
H, DH = 16, 64
NCORES = 8
HPC = H // NCORES          # heads per core = 2
DPC = HPC * DH             # head-dims per core = 128
BT = B * T                 # 4096 tokens
CT = D // 128              # 8 contraction tiles of 128
TPB = T // 128             # 8 key/query 128-tiles per batch row
ROPE_BASE = 10000.0

F32 = mybir.dt.float32
BF16 = mybir.dt.bfloat16
AF = mybir.ActivationFunctionType
ALU = mybir.AluOpType

SWAP_MASK = [i ^ 1 for i in range(32)]  # pair swap within each 32-partition group

_compiled = {}


def _build_nc():
    nc = bacc.Bacc(None, target_bir_lowering=False, debug=False)

    # x packed dense on host: xp[p, ((b*2+th)*CT + ct)*512 + t]
    xp = nc.declare_dram_parameter("xp", [128, B * 2 * CT * 512], BF16,
                                   isOutput=False)
    # qkv weights prepacked on host to [128, CT*128] (SBUF layout, single DMA)
    wq = nc.declare_dram_parameter("wq", [128, CT * DPC], BF16, isOutput=False)
    wk = nc.declare_dram_parameter("wk", [128, CT * DPC], BF16, isOutput=False)
    wv = nc.declare_dram_parameter("wv", [128, CT * DPC], BF16, isOutput=False)
    # wo packed [128, CT*D]: block ct = Wo.T rows [128ct:128(ct+1)] (all 1024 cols)
    wo = nc.declare_dram_parameter("wo", [128, CT * D], BF16, isOutput=False)
    cosb = nc.declare_dram_parameter("cosb", [DPC, T], BF16, isOutput=False)
    sinb = nc.declare_dram_parameter("sinb", [DPC, T], BF16, isOutput=False)
    triu = nc.declare_dram_parameter("triu", [128, 128], BF16, isOutput=False)
    # output [tokens, e]: rows [128b:128(b+1)] = batch row b, my 128 tokens
    yO = nc.declare_dram_parameter("yO", [B * 128, D], F32, isOutput=True)

    with tile.TileContext(nc) as tc:
        with contextlib.ExitStack() as ctx:
            dram = ctx.enter_context(tc.tile_pool(name="dram", bufs=1, space="DRAM"))
            # per-batch-row AllToAll bounce buffers
            ag_in = [dram.tile([D, 128], BF16, name=f"agin{b}") for b in range(B)]
            ag_out = [dram.tile([D, 128], BF16, name=f"agout{b}") for b in range(B)]

            consts = ctx.enter_context(tc.tile_pool(name="consts", bufs=1))

            # qkv weights on the scalar (Activation) DMA ring in unit
            # consumption order (v-unit runs first); cos/sin/triu go on the
            # gpsimd ring so both load in parallel with row 0's x
            cos_sb = consts.tile([DPC, T], BF16)
            sin_sb = consts.tile([DPC, T], BF16)
            triu_sb = consts.tile([128, 128], BF16)
            w_sbs = {}
            for wname, w_dr in (("wv", wv), ("wq", wq), ("wk", wk)):
                w_sb = consts.tile(list(w_dr.shape), BF16, name=f"{wname}_sb")
                nc.scalar.dma_start(w_sb[:], w_dr[:])
                w_sbs[wname] = w_sb
            nc.scalar.dma_start(cos_sb[:], cosb[:])
            nc.scalar.dma_start(sin_sb[:], sinb[:])
            nc.scalar.dma_start(triu_sb[:], triu[:])
            wq_sb, wk_sb, wv_sb = (w_sbs[n] for n in ("wq", "wk", "wv"))
            wo_sb = consts.tile([128, CT * D], BF16, name="wo_sb")

            xpool = ctx.enter_context(tc.tile_pool(name="xTp", bufs=1))
            # x tiles are th-major [128, th, ct, t] so every DMA is fully
            # linear on both sides (host packs xp in exactly this order).
            # Row 0 in 2 half DMAs (the ci=0 QKV groups become runnable after
            # the first 1MB); rows 1-3 one linear 2MB DMA each.
            xrows = []
            xrow0 = xpool.tile([128, 2, CT, 512], BF16, tag="xr0", name="xrow0")
            # half 0 in two ct-quarters so the first QKV group can start
            # streaming after 512KB; half 1 as one linear DMA
            for cq in range(2):
                off = cq * 4 * 512
                nc.sync.dma_start(
                    xrow0[:, 0, cq * 4:cq * 4 + 4, :],
                    xp[:, off:off + 4 * 512].rearrange("p (ct t) -> p ct t",
                                                       t=512))
            xrows = [xrow0]
            for b in range(1, B):
                xrows.append(xpool.tile([128, 2, CT, 512], BF16, tag=f"xr{b}",
                                        name=f"xrow{b}"))
            nc.sync.dma_start(
                xrows[2][:].rearrange("p th ct t -> p (th ct t)"),
                xp[:, 4 * CT * 512:6 * CT * 512])
            # wo (2 MB, needed only by o-proj) goes on the sync ring strictly
            # AFTER the x rows: the ring is FIFO, so it never steals x
            # bandwidth during the startup-critical phase
            nc.sync.dma_start(wo_sb[:], wo[:])

            ident = consts.tile([128, 128], BF16)
            make_identity(nc, ident[:])
            # the other half of x streams on the gpsimd ring (own queue +
            # engine) concurrently with the sync ring; issued after the
            # identity so the PE warm-up burst is never delayed
            nc.gpsimd.dma_start(
                xrow0[:, 1, :, :],
                xp[:, CT * 512:2 * CT * 512].rearrange("p (ct t) -> p ct t",
                                                       t=512))
            for b in (1, 3):
                off = 2 * b * CT * 512
                nc.gpsimd.dma_start(
                    xrows[b][:].rearrange("p th ct t -> p (th ct t)"),
                    xp[:, off:off + 2 * CT * 512])

            pers = ctx.enter_context(tc.tile_pool(name="pers", bufs=1))
            qT_sb = pers.tile([128, BT], BF16)
            kT_sb = pers.tile([128, BT], BF16)
            aoT_sb = pers.tile([128, BT], BF16)
            # persistent [ones | v_h0 | ones | v_h1] PV lhsT tiles, double
            # buffered by row parity; the ones columns are set once
            v_tiles = [[pers.tile([128, 256], BF16, name=f"v{par}_{kt}")
                        for kt in range(TPB)] for par in range(2)]
            for par in range(2):
                for kt in range(TPB):
                    nc.gpsimd.memset(v_tiles[par][kt][:, 0:64], 1.0)
                    nc.gpsimd.memset(v_tiles[par][kt][:, 128:192], 1.0)
            # e-tiles (exp'd scores, [tk, tq] transposed), column c = global
            # query column kt*128 + c; double buffered by row parity
            e_tiles = [{(h, kt): pers.tile([128, T - kt * 128], BF16,
                                           name=f"e{par}_{h}_{kt}")
                        for h in range(HPC) for kt in range(TPB)}
                       for par in range(2)]

            ppool = ctx.enter_context(
                tc.tile_pool(name="proj_psum", bufs=2, space="PSUM"))
            rtp = ctx.enter_context(tc.tile_pool(name="rope_tmp", bufs=2))
            vtmp = ctx.enter_context(tc.tile_pool(name="vtmp", bufs=2))
            spsum = ctx.enter_context(
                tc.tile_pool(name="s_psum", bufs=2, space="PSUM"))
            opsum = ctx.enter_context(
                tc.tile_pool(name="o_psum", bufs=2, space="PSUM"))
            aof_pool = ctx.enter_context(tc.tile_pool(name="aof", bufs=4))
            yout = ctx.enter_context(tc.tile_pool(name="yout", bufs=2))

            scale = float(DH) ** -0.5
            vts_store = {b: {} for b in range(B)}
            aof_store = {}

            def qkv_units(b):
                """Per-(chunk, projection) closures: 8-MM groups + RoPE."""
                units = []
                for ci in range(2):
                    ch = 2 * b + ci
                    sl = slice(ch * 512, ch * 512 + 512)
                    tsl = slice(ci * 512, ci * 512 + 512)

                    def mk_v(b=b, ci=ci, ch=ch):
                        pv = ppool.tile([128, 512], F32, tag="proj",
                                        name=f"pv{ch}")
                        for ct in range(CT):
                            nc.tensor.matmul(
                                pv[:], wv_sb[:, ct * DPC:(ct + 1) * DPC],
                                xrows[b][:, ci, ct, :],
                                start=(ct == 0), stop=(ct == CT - 1))
                        vt = vtmp.tile([128, 512], BF16, tag="vt",
                                       name=f"vt{ch}")
                        nc.vector.tensor_copy(vt[:], pv[:])
                        vts_store[b][ci] = vt
                    units.append(mk_v)

                    for wsb, dst, pname in ((wq_sb, qT_sb, "pq"),
                                            (wk_sb, kT_sb, "pk")):
                        def mk_qk(b=b, ci=ci, ch=ch, sl=sl, tsl=tsl,
                                  wsb=wsb, dst=dst, pname=pname):
                            pp = ppool.tile([128, 512], F32, tag="proj",
                                            name=f"{pname}{ch}")
                            for ct in range(CT):
                                nc.tensor.matmul(
                                    pp[:], wsb[:, ct * DPC:(ct + 1) * DPC],
                                    xrows[b][:, ci, ct, :],
                                    start=(ct == 0), stop=(ct == CT - 1))
                            # stream_shuffle needs an SBUF source; the
                            # cos-mult reads the projection PSUM directly
                            qraw = rtp.tile([128, 512], BF16, tag="qraw",
                                            name=f"qraw{pname}{ch}")
                            nc.scalar.copy(qraw[:], pp[:])
                            sw = rtp.tile([128, 512], BF16, tag="sw",
                                          name=f"sw{pname}{ch}")
                            m1 = rtp.tile([128, 512], BF16, tag="m1",
                                          name=f"m1{pname}{ch}")
                            m2 = rtp.tile([128, 512], BF16, tag="m2",
                                          name=f"m2{pname}{ch}")
                            nc.vector.stream_shuffle(sw[:], qraw[:], SWAP_MASK)
                            nc.vector.tensor_tensor(m1[:], pp[:],
                                                    cos_sb[:, tsl], ALU.mult)
                            nc.vector.tensor_tensor(m2[:], sw[:],
                                                    sin_sb[:, tsl], ALU.mult)
                            nc.vector.tensor_tensor(dst[:, sl], m1[:], m2[:],
                                                    ALU.add)
                        units.append(mk_qk)
                # [v0, q0, v1, k0, q1, k1]: v's early (feed next row's
                # transposes), q_ci1/k_ci1 not last-minute (next row's
                # scores need the full qT/kT row through the RoPE pipeline)
                return [units[i] for i in (0, 1, 3, 2, 4, 5)]

            def transpose_unit(b, kt):
                """[d, t] -> v_tiles[par][kt] [ones|v_h0|ones|v_h1] columns."""
                vts = vts_store[b]
                vt = v_tiles[b % 2]
                pt = ppool.tile([128, 128], BF16, tag="proj", name=f"pt{b}_{kt}")
                nc.tensor.transpose(pt[:], vts[kt // 4][:, (kt % 4) * 128:
                                                        (kt % 4) * 128 + 128],
                                    ident[:])
                nc.scalar.copy(vt[kt][:, 64:128], pt[:, 0:64])
                nc.vector.tensor_copy(vt[kt][:, 192:256], pt[:, 64:128])

            def attention(b, fillers):
                """Scores+exp+PV+normalize for row b; fillers (next-row QKV
                units) are emitted between groups so the PE queue always has
                ready matmuls behind an exp-gated attention group."""
                b0 = b * T
                par = b % 2
                vt = v_tiles[par]
                et = e_tiles[par]
                nfill = len(fillers)
                emitted = 0
                point = 0

                def fill_point():
                    nonlocal emitted, point
                    point += 1
                    target = min(nfill, (point * nfill) // 12)
                    while emitted < target:
                        fillers[emitted]()
                        emitted += 1

                for kt in range(TPB):
                    transpose_unit(b, kt)
                    lo = kt * 128
                    for h in range(HPC):
                        hsl = slice(h * 64, (h + 1) * 64)
                        ps = spsum.tile([128, T], F32, tag="s",
                                        name=f"s{b}_{h}_{kt}")
                        # bank-aligned score matmuls over the valid range only
                        if lo < 512:
                            nc.tensor.matmul(ps[:, lo:512],
                                             kT_sb[hsl, b0 + lo:b0 + lo + 128],
                                             qT_sb[hsl, b0 + lo:b0 + 512],
                                             start=True, stop=True)
                        nc.tensor.matmul(ps[:, max(lo, 512):T],
                                         kT_sb[hsl, b0 + lo:b0 + lo + 128],
                                         qT_sb[hsl, b0 + max(lo, 512):b0 + T],
                                         start=True, stop=True)
                        nc.scalar.activation(et[(h, kt)][:], ps[:, lo:T],
                                             AF.Exp, scale=scale)
                        nc.vector.tensor_tensor(
                            et[(h, kt)][:, 0:128], et[(h, kt)][:, 0:128],
                            triu_sb[:], ALU.mult)
                    fill_point()
                for half in range(2):
                    c0 = half * 512
                    for h in range(HPC):
                        # lhsT = [ones | v_h]: PSUM rows 0:64 = denom (at base
                        # partition 0, which the custom-DVE reciprocal
                        # requires), rows 64:128 = PV.
                        po = opsum.tile([128, 512], F32, tag="po",
                                        name=f"po{b}_{h}_{half}")
                        nkt = TPB if half else 4
                        for kt in range(nkt):
                            lo = kt * 128
                            s0 = max(c0 - lo, 0)
                            nc.tensor.matmul(
                                po[:, max(lo - c0, 0):512],
                                vt[kt][:, h * 128:h * 128 + 128],
                                et[(h, kt)][:, s0:c0 + 512 - lo],
                                start=(kt == 0), stop=(kt == nkt - 1))
                        den = rtp.tile([64, 512], F32, tag="den",
                                       name=f"den{b}_{h}_{half}")
                        nc.vector.reciprocal_approx_fast(den[:], po[0:64, :])
                        nc.vector.tensor_tensor(
                            aoT_sb[h * 64:(h + 1) * 64, b0 + c0:b0 + c0 + 512],
                            po[64:128, :], den[:], ALU.mult)
                        fill_point()
                    # bounce this tq-half to the A2A input as soon as both
                    # heads' normalize is done: halves the bounce latency on
                    # the collective-trigger path
                    src = aoT_sb[:, b0 + c0:b0 + c0 + 512].rearrange(
                        "c (j q) -> c j q", q=128)
                    dst = ag_in[b][:].rearrange(
                        "(j c) q -> c j q", c=128)[:, half * 4:half * 4 + 4, :]
                    nc.gpsimd.dma_start(dst, src)
                while emitted < nfill:
                    fillers[emitted]()
                    emitted += 1

            def alltoall(b):
                # shard-major bounce (already in ag_in): A2A swaps shards so
                # ag_out stacks all ranks' head-dim blocks for MY tokens
                nc.gpsimd.collective_compute(
                    "AllToAll", ALU.bypass,
                    replica_groups=[list(range(NCORES))],
                    ins=[ag_in[b][:]], outs=[ag_out[b][:]])

            def aof_load(b, chunks, rings):
                """Gather ag_out[b] -> aof SBUF tile. dma_start BLOCKS its
                engine until A2A(b)'s completion semaphore is live, so each
                load sits on an engine with nothing useful left to do."""
                aof = aof_pool.tile([128, CT * 128], BF16, tag="aof",
                                    name=f"aof{b}")
                n = CT // chunks
                for c in range(chunks):
                    ring = rings[c % len(rings)]
                    ring.dma_start(
                        aof[:, c * n * 128:(c + 1) * n * 128].rearrange(
                            "c (ct q) -> c ct q", ct=n),
                        ag_out[b][c * n * 128:(c + 1) * n * 128, :].rearrange(
                            "(ct c) q -> c ct q", c=128))
                aof_store[b] = aof

            def oproj_units(b, last=False):
                """Token-stationary o-proj: y[t, e] for my 128 tokens of row
                b, as two 8-MM chain units (one per 512-wide output half)
                with the PSUM drain + output DMA inside the unit, so the
                first half's copy/DMA overlaps the second half's matmuls.
                All drains on the vector engine: at the tail the scalar
                engine sits blocked behind pre-issued aof gathers."""
                def mk(hf, b=b):
                    aof = aof_store[b]
                    yp = opsum.tile([128, 512], F32, tag="po",
                                    name=f"yp{b}_{hf}")
                    for ct in range(CT):
                        nc.tensor.matmul(
                            yp[:],
                            aof[:, ct * 128:(ct + 1) * 128],
                            wo_sb[:, ct * D + hf * 512:ct * D + hf * 512 + 512],
                            start=(ct == 0), stop=(ct == CT - 1))
                    if last and hf:
                        # chunked drain on two rings: first 256-col DMA
                        # overlaps the second copy, shortening the tail
                        for q, ring in ((0, nc.sync), (1, nc.gpsimd)):
                            yo = yout.tile([128, 256], F32, tag=f"yoq{q}",
                                           name=f"yo{b}_{hf}_{q}")
                            nc.vector.tensor_copy(yo[:],
                                                  yp[:, q * 256:q * 256 + 256])
                            ring.dma_start(
                                yO[b * 128:(b + 1) * 128,
                                   hf * 512 + q * 256:hf * 512 + q * 256 + 256],
                                yo[:])
                        return
                    yo = yout.tile([128, 512], F32, tag=f"yo{hf}",
                                   name=f"yo{b}_{hf}")
                    nc.vector.tensor_copy(yo[:], yp[:])
                    nc.sync.dma_start(
                        yO[b * 128:(b + 1) * 128, hf * 512:hf * 512 + 512],
                        yo[:])
                return [lambda hf=hf: mk(hf) for hf in range(2)]

            # PE warm-up burst: dep-free identity matmuls run back-to-back
            # the moment the identity is built (~8us). The HAM SHORT window
            # needs ~3.4us of SUSTAINED activity to unthrottle; the burst
            # crosses that window and drains about when the first x half
            # lands (~10.5us with the dense pack), so it never delays the
            # row-0 projections.
            wps = ppool.tile([128, 512], F32, tag="proj", name="warm")
            for i in range(32):
                nc.tensor.matmul(wps[:, 0:128], ident[:], ident[:],
                                 start=True, stop=True)
            # one matmul chained to each x half keeps the gate open while the
            # rest of row 0 streams in
            for th in range(2):
                nc.tensor.matmul(wps[:, 0:128], ident[:],
                                 xrows[0][:, th, 0, 0:128],
                                 start=True, stop=True)

            for u in qkv_units(0):
                u()
            for b in range(B):
                fillers = qkv_units(b + 1) if b + 1 < B else []
                attention(b, fillers)
                alltoall(b)
                if b == 2:
                    # A2A(0)/A2A(1) are done (or nearly) by now, so these
                    # blocking gathers cost the sync ring nothing
                    aof_load(0, 2, [nc.sync])
                    aof_load(1, 2, [nc.sync])
            # ALL o-proj compute sits in the A2A(3) straggler-wait window;
            # everything oproj(0..2) reads is local by now, so the in-order
            # PE queue cannot head-of-line block on a late peer. No junk
            # keeper before oproj(3): a clean idle gap earns a HAM 8/8
            # grant, while sparse keep-alive activity pins the 4/8 window.
            for u in oproj_units(0):
                u()
            for u in oproj_units(1):
                u()
            aof_load(2, 8, [nc.sync, nc.scalar])
            for u in oproj_units(2):
                u()
            aof_load(3, 8, [nc.gpsimd, nc.scalar])
            for u in oproj_units(3, last=True):
                u()

    nc.compile()
    return nc


def _host_inputs(x, Wq, Wk, Wv, Wo):
    bf16 = ml_dtypes.bfloat16
    x2 = np.asarray(x, dtype=np.float32).reshape(BT, D)
    # dense pack: xp[p, ((b*2+th)*CT + ct)*512 + t] = x2.T[ct*128+p, b*1024+th*512+t]
    xT = x2.T.reshape(CT, 128, B, 2, 512)
    xpk = np.ascontiguousarray(
        xT.transpose(1, 2, 3, 0, 4).reshape(128, B * 2 * CT * 512)).astype(bf16)

    inv_freq = 1.0 / (ROPE_BASE ** (np.arange(0, DH, 2, dtype=np.float32) / DH))
    tpos = np.arange(T, dtype=np.float32)
    freqs = np.outer(tpos, inv_freq).astype(np.float32)   # [T, 32]
    cos = np.cos(freqs).astype(np.float32)
    sin = np.sin(freqs).astype(np.float32)
    pidx = (np.arange(DPC) % DH) // 2
    cosb = np.ascontiguousarray(cos.T[pidx, :]).astype(np.float32)  # [128, T]
    sign = np.where(np.arange(DPC) % 2 == 0, -1.0, 1.0).astype(np.float32)
    sinb = np.ascontiguousarray(sin.T[pidx, :] * sign[:, None]).astype(np.float32)

    triu = np.triu(np.ones((128, 128), np.float32)).astype(bf16)

    def prepack(W, i):
        sl = slice(i * DPC, (i + 1) * DPC)
        wT = np.asarray(W, np.float32)[sl, :].T          # [1024, 128]
        blocks = [wT[ct * 128:(ct + 1) * 128, :] for ct in range(CT)]
        return np.ascontiguousarray(np.concatenate(blocks, axis=1)).astype(bf16)

    # wo packed [128, CT*D]: block ct = Wo.T rows [128ct:128(ct+1)]
    woT = np.ascontiguousarray(np.asarray(Wo, np.float32).T)   # [c, e]
    wo_blocks = [woT[ct * 128:(ct + 1) * 128, :] for ct in range(CT)]
    wo_packed = np.ascontiguousarray(np.concatenate(wo_blocks, axis=1)).astype(bf16)

    in_maps = []
    for i in range(NCORES):
        m = {
            "xp": xpk,
            "wq": prepack(Wq, i),
            "wk": prepack(Wk, i),
            "wv": prepack(Wv, i),
            "wo": wo_packed,
            "cosb": cosb.astype(bf16),
            "sinb": sinb.astype(bf16),
            "triu": triu,
        }
        in_maps.append(m)
    return in_maps


def kernel(x, Wq, Wk, Wv, Wo, _trace=False):
    if "nc" not in _compiled:
        _compiled["nc"] = _build_nc()
    nc = _compiled["nc"]
    in_maps = _host_inputs(x, Wq, Wk, Wv, Wo)
    res = run_bass_kernel_spmd(nc, in_maps, list(range(NCORES)), trace=_trace)
    _compiled["last_result"] = res
    # core j holds yO_j [512, 1024]: rows [128b:128(b+1)) = batch row b,
    # tokens [128j:128(j+1)), full 1024 output dims
    y = np.empty((B, T, D), np.float32)
    for j in range(NCORES):
        yo = res.results[j]["yO"]           # [512, 1024]
        for b in range(B):
            y[b, 128 * j:128 * (j + 1), :] = yo[128 * b:128 * (b + 1), :]
    return y


# revision 19
# speedup vs baseline: 1.0466x; 1.0466x over previous
"""Multi-head causal attention (RoPE) on 8 TRN2 NeuronCores.

Sharding: tensor-parallel over heads. Each core computes 2 of the 16 heads:
column-parallel q/k/v projections, local attention, then a per-batch-row
AllToAll of the transposed attention outputs and a token-parallel o-proj
(each core produces the full 1024-wide output for 128 tokens per row).

Layout strategy: activations live transposed on-chip ([dim, token]) so every
matmul contracts over the partition axis with no transposes of x. Scores are
computed transposed ([tk, tq]); softmax has no max-subtraction (logits are
O(1) for this input distribution) and its denominator is produced by a
64-wide ones block appended to V in the PV matmul; normalization is a single
tensor-tensor divide per (b, head, tq-half) writing bf16 aoT directly.
RoPE uses the interleaved-pair identity q' = q*C + swap(q)*S', with the pair
swap done by the DVE stream-shuffle.

e-tiles and v-tiles are double-buffered by row parity so next-row filler
writes never WAR-stall against the current row's PV reads.

o-proj is token-stationary: after the per-row AllToAll each core holds all
1024 attention dims for its 128 tokens of that row; the 128-token tile is the
matmul stationary operand and Wo.T streams as the moving operand (N=512).

Schedule: attention(b) is emitted with next-row QKV projection units
INTERLEAVED between its score/PV groups (keeps the in-order PE queue fed
behind exp-gated groups). The aoT->ag_in bounce is split per tq-half and
issued as each half's normalize completes, pulling the collective trigger
earlier. ALL o-proj compute is deferred until after the A2A(3) trigger so
the last collective's straggler wait is filled with oproj(0..2); oproj(3)
starts right after A2A(3) lands via per-ct aof gathers split across two
otherwise-idle DMA rings. dma_start BLOCKS its issuing engine until the
transfer's input semaphore is live, so every aof gather is placed on an
engine/queue position where that block is harmless, and all yo drains run
on the vector engine (scalar sits blocked behind aof gathers at the tail).
x is host-packed dense and th-major on SBUF so every x DMA is a fully
linear 8-16KB-per-partition transfer.
"""

import sys

for _p in ("/opt/trn_rl_repo",):
    if _p not in sys.path:
        sys.path.insert(0, _p)

import contextlib

import numpy as np
import ml_dtypes

import concourse.bass as bass
import concourse.mybir as mybir
import concourse.tile as tile
from concourse import bacc
from concourse.bass_utils import run_bass_kernel_spmd
from concourse.masks import make_identity

# Problem constants (nn_MultiHeadAttention: x [4,1024,1024], 16 heads)
B, T, D = 4, 1024, 1024
H, DH = 16, 64
NCORES = 8
HPC = H // NCORES          # heads per core = 2
DPC = HPC * DH             # head-dims per core = 128
BT = B * T                 # 4096 tokens
CT = D // 128              # 8 contraction tiles of 128
TPB = T // 128             # 8 key/query 128-tiles per batch row
ROPE_BASE = 10000.0

F32 = mybir.dt.float32
BF16 = mybir.dt.bfloat16
AF = mybir.ActivationFunctionType
ALU = mybir.AluOpType

SWAP_MASK = [i ^ 1 for i in range(32)]  # pair swap within each 32-partition group

_compiled = {}


def _build_nc():
    nc = bacc.Bacc(None, target_bir_lowering=False, debug=False)

    # x packed dense on host: xp[p, ((b*2+th)*CT + ct)*512 + t]
    xp = nc.declare_dram_parameter("xp", [128, B * 2 * CT * 512], BF16,
                                   isOutput=False)
    # qkv weights prepacked on host to [128, CT*128] (SBUF layout, single DMA)
    wq = nc.declare_dram_parameter("wq", [128, CT * DPC], BF16, isOutput=False)
    wk = nc.declare_dram_parameter("wk", [128, CT * DPC], BF16, isOutput=False)
    wv = nc.declare_dram_parameter("wv", [128, CT * DPC], BF16, isOutput=False)
    # wo packed [128, CT*D]: block ct = Wo.T rows [128ct:128(ct+1)] (all 1024 cols)
    wo = nc.declare_dram_parameter("wo", [128, CT * D], BF16, isOutput=False)
    cosb = nc.declare_dram_parameter("cosb", [DPC, T], BF16, isOutput=False)
    sinb = nc.declare_dram_parameter("sinb", [DPC, T], BF16, isOutput=False)
    triu = nc.declare_dram_parameter("triu", [128, 128], BF16, isOutput=False)
    # output [tokens, e]: rows [128b:128(b+1)] = batch row b, my 128 tokens
    yO = nc.declare_dram_parameter("yO", [B * 128, D], F32, isOutput=True)

    with tile.TileContext(nc) as tc:
        with contextlib.ExitStack() as ctx:
            dram = ctx.enter_context(tc.tile_pool(name="dram", bufs=1, space="DRAM"))
            # per-batch-row AllToAll bounce buffers
            ag_in = [dram.tile([D, 128], BF16, name=f"agin{b}") for b in range(B)]
            ag_out = [dram.tile([D, 128], BF16, name=f"agout{b}") for b in range(B)]

            consts = ctx.enter_context(tc.tile_pool(name="consts", bufs=1))

            # qkv weights on the scalar (Activation) DMA ring in unit
            # consumption order (v-unit runs first); cos/sin/triu go on the
            # gpsimd ring so both load in parallel with row 0's x
            cos_sb = consts.tile([DPC, T], BF16)
            sin_sb = consts.tile([DPC, T], BF16)
            triu_sb = consts.tile([128, 128], BF16)
            w_sbs = {}
            for wname, w_dr in (("wv", wv), ("wq", wq), ("wk", wk)):
                w_sb = consts.tile(list(w_dr.shape), BF16, name=f"{wname}_sb")
                nc.scalar.dma_start(w_sb[:], w_dr[:])
                w_sbs[wname] = w_sb
            nc.scalar.dma_start(cos_sb[:], cosb[:])
            nc.scalar.dma_start(sin_sb[:], sinb[:])
            nc.scalar.dma_start(triu_sb[:], triu[:])
            wq_sb, wk_sb, wv_sb = (w_sbs[n] for n in ("wq", "wk", "wv"))
            wo_sb = consts.tile([128, CT * D], BF16, name="wo_sb")

            xpool = ctx.enter_context(tc.tile_pool(name="xTp", bufs=1))
            # x tiles are th-major [128, th, ct, t] so every DMA is fully
            # linear on both sides (host packs xp in exactly this order).
            # Row 0 in 2 half DMAs (the ci=0 QKV groups become runnable after
            # the first 1MB); rows 1-3 one linear 2MB DMA each.
            xrows = []
            xrow0 = xpool.tile([128, 2, CT, 512], BF16, tag="xr0", name="xrow0")
            # half 0 in two ct-quarters so the first QKV group can start
            # streaming after 512KB; half 1 as one linear DMA
            for cq in range(2):
                off = cq * 4 * 512
                nc.sync.dma_start(
                    xrow0[:, 0, cq * 4:cq * 4 + 4, :],
                    xp[:, off:off + 4 * 512].rearrange("p (ct t) -> p ct t",
                                                       t=512))
            nc.sync.dma_start(
                xrow0[:, 1, :, :],
                xp[:, CT * 512:2 * CT * 512].rearrange("p (ct t) -> p ct t",
                                                       t=512))
            xrows = [xrow0]
            for b in range(1, B):
                xrow = xpool.tile([128, 2, CT, 512], BF16, tag=f"xr{b}",
                                  name=f"xrow{b}")
                off = 2 * b * CT * 512
                nc.sync.dma_start(
                    xrow[:].rearrange("p th ct t -> p (th ct t)"),
                    xp[:, off:off + 2 * CT * 512])
                xrows.append(xrow)
            # wo (2 MB, needed only by o-proj) goes on the sync ring strictly
            # AFTER the x rows: the ring is FIFO, so it never steals x
            # bandwidth during the startup-critical phase
            nc.sync.dma_start(wo_sb[:], wo[:])

            ident = consts.tile([128, 128], BF16)
            make_identity(nc, ident[:])

            pers = ctx.enter_context(tc.tile_pool(name="pers", bufs=1))
            qT_sb = pers.tile([128, BT], BF16)
            kT_sb = pers.tile([128, BT], BF16)
            aoT_sb = pers.tile([128, BT], BF16)
            # persistent [ones | v_h0 | ones | v_h1] PV lhsT tiles, double
            # buffered by row parity; the ones columns are set once
            v_tiles = [[pers.tile([128, 256], BF16, name=f"v{par}_{kt}")
                        for kt in range(TPB)] for par in range(2)]
            for par in range(2):
                for kt in range(TPB):
                    nc.gpsimd.memset(v_tiles[par][kt][:, 0:64], 1.0)
                    nc.gpsimd.memset(v_tiles[par][kt][:, 128:192], 1.0)
            # e-tiles (exp'd scores, [tk, tq] transposed), column c = global
            # query column kt*128 + c; double buffered by row parity
            e_tiles = [{(h, kt): pers.tile([128, T - kt * 128], BF16,
                                           name=f"e{par}_{h}_{kt}")
                        for h in range(HPC) for kt in range(TPB)}
                       for par in range(2)]

            ppool = ctx.enter_context(
                tc.tile_pool(name="proj_psum", bufs=2, space="PSUM"))
            rtp = ctx.enter_context(tc.tile_pool(name="rope_tmp", bufs=2))
            vtmp = ctx.enter_context(tc.tile_pool(name="vtmp", bufs=2))
            spsum = ctx.enter_context(
                tc.tile_pool(name="s_psum", bufs=2, space="PSUM"))
            opsum = ctx.enter_context(
                tc.tile_pool(name="o_psum", bufs=2, space="PSUM"))
            aof_pool = ctx.enter_context(tc.tile_pool(name="aof", bufs=4))
            yout = ctx.enter_context(tc.tile_pool(name="yout", bufs=2))

            scale = float(DH) ** -0.5
            vts_store = {b: {} for b in range(B)}
            aof_store = {}

            def qkv_units(b):
                """Per-(chunk, projection) closures: 8-MM groups + RoPE."""
                units = []
                for ci in range(2):
                    ch = 2 * b + ci
                    sl = slice(ch * 512, ch * 512 + 512)
                    tsl = slice(ci * 512, ci * 512 + 512)

                    def mk_v(b=b, ci=ci, ch=ch):
                        pv = ppool.tile([128, 512], F32, tag="proj",
                                        name=f"pv{ch}")
                        for ct in range(CT):
                            nc.tensor.matmul(
                                pv[:], wv_sb[:, ct * DPC:(ct + 1) * DPC],
                                xrows[b][:, ci, ct, :],
                                start=(ct == 0), stop=(ct == CT - 1))
                        vt = vtmp.tile([128, 512], BF16, tag="vt",
                                       name=f"vt{ch}")
                        nc.vector.tensor_copy(vt[:], pv[:])
                        vts_store[b][ci] = vt
                    units.append(mk_v)

                    for wsb, dst, pname in ((wq_sb, qT_sb, "pq"),
                                            (wk_sb, kT_sb, "pk")):
                        def mk_qk(b=b, ci=ci, ch=ch, sl=sl, tsl=tsl,
                                  wsb=wsb, dst=dst, pname=pname):
                            pp = ppool.tile([128, 512], F32, tag="proj",
                                            name=f"{pname}{ch}")
                            for ct in range(CT):
                                nc.tensor.matmul(
                                    pp[:], wsb[:, ct * DPC:(ct + 1) * DPC],
                                    xrows[b][:, ci, ct, :],
                                    start=(ct == 0), stop=(ct == CT - 1))
                            # stream_shuffle needs an SBUF source; the
                            # cos-mult reads the projection PSUM directly
                            qraw = rtp.tile([128, 512], BF16, tag="qraw",
                                            name=f"qraw{pname}{ch}")
                            nc.scalar.copy(qraw[:], pp[:])
                            sw = rtp.tile([128, 512], BF16, tag="sw",
                                          name=f"sw{pname}{ch}")
                            m1 = rtp.tile([128, 512], BF16, tag="m1",
                                          name=f"m1{pname}{ch}")
                            m2 = rtp.tile([128, 512], BF16, tag="m2",
                                          name=f"m2{pname}{ch}")
                            nc.vector.stream_shuffle(sw[:], qraw[:], SWAP_MASK)
                            nc.vector.tensor_tensor(m1[:], pp[:],
                                                    cos_sb[:, tsl], ALU.mult)
                            nc.vector.tensor_tensor(m2[:], sw[:],
                                                    sin_sb[:, tsl], ALU.mult)
                            nc.vector.tensor_tensor(dst[:, sl], m1[:], m2[:],
                                                    ALU.add)
                        units.append(mk_qk)
                # [v0, q0, v1, k0, q1, k1]: v's early (feed next row's
                # transposes), q_ci1/k_ci1 not last-minute (next row's
                # scores need the full qT/kT row through the RoPE pipeline)
                return [units[i] for i in (0, 1, 3, 2, 4, 5)]

            def transpose_unit(b, kt):
                """[d, t] -> v_tiles[par][kt] [ones|v_h0|ones|v_h1] columns."""
                vts = vts_store[b]
                vt = v_tiles[b % 2]
                pt = ppool.tile([128, 128], BF16, tag="proj", name=f"pt{b}_{kt}")
                nc.tensor.transpose(pt[:], vts[kt // 4][:, (kt % 4) * 128:
                                                        (kt % 4) * 128 + 128],
                                    ident[:])
                nc.scalar.copy(vt[kt][:, 64:128], pt[:, 0:64])
                nc.vector.tensor_copy(vt[kt][:, 192:256], pt[:, 64:128])

            def attention(b, fillers):
                """Scores+exp+PV+normalize for row b; fillers (next-row QKV
                units) are emitted between groups so the PE queue always has
                ready matmuls behind an exp-gated attention group."""
                b0 = b * T
                par = b % 2
                vt = v_tiles[par]
                et = e_tiles[par]
                nfill = len(fillers)
                emitted = 0
                point = 0

                def fill_point():
                    nonlocal emitted, point
                    point += 1
                    target = min(nfill, (point * nfill) // 12)
                    while emitted < target:
                        fillers[emitted]()
                        emitted += 1

                for kt in range(TPB):
                    transpose_unit(b, kt)
                    lo = kt * 128
                    for h in range(HPC):
                        hsl = slice(h * 64, (h + 1) * 64)
                        ps = spsum.tile([128, T], F32, tag="s",
                                        name=f"s{b}_{h}_{kt}")
                        # bank-aligned score matmuls over the valid range only
                        if lo < 512:
                            nc.tensor.matmul(ps[:, lo:512],
                                             kT_sb[hsl, b0 + lo:b0 + lo + 128],
                                             qT_sb[hsl, b0 + lo:b0 + 512],
                                             start=True, stop=True)
                        nc.tensor.matmul(ps[:, max(lo, 512):T],
                                         kT_sb[hsl, b0 + lo:b0 + lo + 128],
                                         qT_sb[hsl, b0 + max(lo, 512):b0 + T],
                                         start=True, stop=True)
                        nc.scalar.activation(et[(h, kt)][:], ps[:, lo:T],
                                             AF.Exp, scale=scale)
                        nc.vector.tensor_tensor(
                            et[(h, kt)][:, 0:128], et[(h, kt)][:, 0:128],
                            triu_sb[:], ALU.mult)
                    fill_point()
                for half in range(2):
                    c0 = half * 512
                    for h in range(HPC):
                        # lhsT = [ones | v_h]: PSUM rows 0:64 = denom (at base
                        # partition 0, which the custom-DVE reciprocal
                        # requires), rows 64:128 = PV.
                        po = opsum.tile([128, 512], F32, tag="po",
                                        name=f"po{b}_{h}_{half}")
                        nkt = TPB if half else 4
                        for kt in range(nkt):
                            lo = kt * 128
                            s0 = max(c0 - lo, 0)
                            nc.tensor.matmul(
                                po[:, max(lo - c0, 0):512],
                                vt[kt][:, h * 128:h * 128 + 128],
                                et[(h, kt)][:, s0:c0 + 512 - lo],
                                start=(kt == 0), stop=(kt == nkt - 1))
                        den = rtp.tile([64, 512], F32, tag="den",
                                       name=f"den{b}_{h}_{half}")
                        nc.vector.reciprocal_approx_fast(den[:], po[0:64, :])
                        nc.vector.tensor_tensor(
                            aoT_sb[h * 64:(h + 1) * 64, b0 + c0:b0 + c0 + 512],
                            po[64:128, :], den[:], ALU.mult)
                        fill_point()
                    # bounce this tq-half to the A2A input as soon as both
                    # heads' normalize is done: halves the bounce latency on
                    # the collective-trigger path
                    src = aoT_sb[:, b0 + c0:b0 + c0 + 512].rearrange(
                        "c (j q) -> c j q", q=128)
                    dst = ag_in[b][:].rearrange(
                        "(j c) q -> c j q", c=128)[:, half * 4:half * 4 + 4, :]
                    nc.gpsimd.dma_start(dst, src)
                while emitted < nfill:
                    fillers[emitted]()
                    emitted += 1

            def alltoall(b):
                # shard-major bounce (already in ag_in): A2A swaps shards so
                # ag_out stacks all ranks' head-dim blocks for MY tokens
                nc.gpsimd.collective_compute(
                    "AllToAll", ALU.bypass,
                    replica_groups=[list(range(NCORES))],
                    ins=[ag_in[b][:]], outs=[ag_out[b][:]])

            def aof_load(b, chunks, rings):
                """Gather ag_out[b] -> aof SBUF tile. dma_start BLOCKS its
                engine until A2A(b)'s completion semaphore is live, so each
                load sits on an engine with nothing useful left to do."""
                aof = aof_pool.tile([128, CT * 128], BF16, tag="aof",
                                    name=f"aof{b}")
                n = CT // chunks
                for c in range(chunks):
                    ring = rings[c % len(rings)]
                    ring.dma_start(
                        aof[:, c * n * 128:(c + 1) * n * 128].rearrange(
                            "c (ct q) -> c ct q", ct=n),
                        ag_out[b][c * n * 128:(c + 1) * n * 128, :].rearrange(
                            "(ct c) q -> c ct q", c=128))
                aof_store[b] = aof

            def oproj_units(b, last=False):
                """Token-stationary o-proj: y[t, e] for my 128 tokens of row
                b, as two 8-MM chain units (one per 512-wide output half)
                with the PSUM drain + output DMA inside the unit, so the
                first half's copy/DMA overlaps the second half's matmuls.
                All drains on the vector engine: at the tail the scalar
                engine sits blocked behind pre-issued aof gathers."""
                def mk(hf, b=b):
                    aof = aof_store[b]
                    yp = opsum.tile([128, 512], F32, tag="po",
                                    name=f"yp{b}_{hf}")
                    for ct in range(CT):
                        nc.tensor.matmul(
                            yp[:],
                            aof[:, ct * 128:(ct + 1) * 128],
                            wo_sb[:, ct * D + hf * 512:ct * D + hf * 512 + 512],
                            start=(ct == 0), stop=(ct == CT - 1))
                    if last and hf:
                        # chunked drain on two rings: first 256-col DMA
                        # overlaps the second copy, shortening the tail
                        for q, ring in ((0, nc.sync), (1, nc.gpsimd)):
                            yo = yout.tile([128, 256], F32, tag=f"yoq{q}",
                                           name=f"yo{b}_{hf}_{q}")
                            nc.vector.tensor_copy(yo[:],
                                                  yp[:, q * 256:q * 256 + 256])
                            ring.dma_start(
                                yO[b * 128:(b + 1) * 128,
                                   hf * 512 + q * 256:hf * 512 + q * 256 + 256],
                                yo[:])
                        return
                    yo = yout.tile([128, 512], F32, tag=f"yo{hf}",
                                   name=f"yo{b}_{hf}")
                    nc.vector.tensor_copy(yo[:], yp[:])
                    nc.sync.dma_start(
                        yO[b * 128:(b + 1) * 128, hf * 512:hf * 512 + 512],
                        yo[:])
                return [lambda hf=hf: mk(hf) for hf in range(2)]

            # PE warm-up burst: dep-free identity matmuls run back-to-back
            # the moment the identity is built (~8us). The HAM SHORT window
            # needs ~3.4us of SUSTAINED activity to unthrottle; the burst
            # crosses that window and drains about when the first x half
            # lands (~10.5us with the dense pack), so it never delays the
            # row-0 projections.
            wps = ppool.tile([128, 512], F32, tag="proj", name="warm")
            for i in range(32):
                nc.tensor.matmul(wps[:, 0:128], ident[:], ident[:],
                                 start=True, stop=True)
            # one matmul chained to each x half keeps the gate open while the
            # rest of row 0 streams in
            for th in range(2):
                nc.tensor.matmul(wps[:, 0:128], ident[:],
                                 xrows[0][:, th, 0, 0:128],
                                 start=True, stop=True)

            for u in qkv_units(0):
                u()
            for b in range(B):
                fillers = qkv_units(b + 1) if b + 1 < B else []
                attention(b, fillers)
                alltoall(b)
                if b == 2:
                    # A2A(0)/A2A(1) are done (or nearly) by now, so these
                    # blocking gathers cost the sync ring nothing
                    aof_load(0, 2, [nc.sync])
                    aof_load(1, 2, [nc.sync])
            # ALL o-proj compute sits in the A2A(3) straggler-wait window;
            # everything oproj(0..2) reads is local by now, so the in-order
            # PE queue cannot head-of-line block on a late peer. No junk
            # keeper before oproj(3): a clean idle gap earns a HAM 8/8
            # grant, while sparse keep-alive activity pins the 4/8 window.
            for u in oproj_units(0):
                u()
            for u in oproj_units(1):
                u()
            aof_load(2, 8, [nc.sync, nc.scalar])
            for u in oproj_units(2):
                u()
            aof_load(3, 8, [nc.gpsimd, nc.scalar])
            for u in oproj_units(3, last=True):
                u()

    nc.compile()
    return nc


def _host_inputs(x, Wq, Wk, Wv, Wo):
    bf16 = ml_dtypes.bfloat16
    x2 = np.asarray(x, dtype=np.float32).reshape(BT, D)
    # dense pack: xp[p, ((b*2+th)*CT + ct)*512 + t] = x2.T[ct*128+p, b*1024+th*512+t]
    xT = x2.T.reshape(CT, 128, B, 2, 512)
    xpk = np.ascontiguousarray(
        xT.transpose(1, 2, 3, 0, 4).reshape(128, B * 2 * CT * 512)).astype(bf16)

    inv_freq = 1.0 / (ROPE_BASE ** (np.arange(0, DH, 2, dtype=np.float32) / DH))
    tpos = np.arange(T, dtype=np.float32)
    freqs = np.outer(tpos, inv_freq).astype(np.float32)   # [T, 32]
    cos = np.cos(freqs).astype(np.float32)
    sin = np.sin(freqs).astype(np.float32)
    pidx = (np.arange(DPC) % DH) // 2
    cosb = np.ascontiguousarray(cos.T[pidx, :]).astype(np.float32)  # [128, T]
    sign = np.where(np.arange(DPC) % 2 == 0, -1.0, 1.0).astype(np.float32)
    sinb = np.ascontiguousarray(sin.T[pidx, :] * sign[:, None]).astype(np.float32)

    triu = np.triu(np.ones((128, 128), np.float32)).astype(bf16)

    def prepack(W, i):
        sl = slice(i * DPC, (i + 1) * DPC)
        wT = np.asarray(W, np.float32)[sl, :].T          # [1024, 128]
        blocks = [wT[ct * 128:(ct + 1) * 128, :] for ct in range(CT)]
        return np.ascontiguousarray(np.concatenate(blocks, axis=1)).astype(bf16)

    # wo packed [128, CT*D]: block ct = Wo.T rows [128ct:128(ct+1)]
    woT = np.ascontiguousarray(np.asarray(Wo, np.float32).T)   # [c, e]
    wo_blocks = [woT[ct * 128:(ct + 1) * 128, :] for ct in range(CT)]
    wo_packed = np.ascontiguousarray(np.concatenate(wo_blocks, axis=1)).astype(bf16)

    in_maps = []
    for i in range(NCORES):
        m = {
            "xp": xpk,
            "wq": prepack(Wq, i),
            "wk": prepack(Wk, i),
            "wv": prepack(Wv, i),
            "wo": wo_packed,
            "cosb": cosb.astype(bf16),
            "sinb": sinb.astype(bf16),
            "triu": triu,
        }
        in_maps.append(m)
    return in_maps


def kernel(x, Wq, Wk, Wv, Wo, _trace=False):
    if "nc" not in _compiled:
        _compiled["nc"] = _build_nc()
    nc = _compiled["nc"]
    in_maps = _host_inputs(x, Wq, Wk, Wv, Wo)
    res = run_bass_kernel_spmd(nc, in_maps, list(range(NCORES)), trace=_trace)
    _compiled["last_result"] = res
    # core j holds yO_j [512, 1024]: rows [128b:128(b+1)) = batch row b,
    # tokens [128j:128(j+1)), full 1024 output dims
    y = np.empty((B, T, D), np.float32)
    for j in range(NCORES):
        yo = res.results[j]["yO"]           # [512, 1024]
        for b in range(B):
            y[b, 128 * j:128 * (j + 1), :] = yo[128 * b:128 * (b + 1), :]
    return y
